# Initial kernel scaffold
#
"""NeRF loss kernel for 8 Trainium2 NeuronCores.

Returns (d_rgb, d_depth, d_opacity, d_distortion), each (65536,) f32, matching
the reference:
  d_rgb        = mean((rgb_coarse-rgb_target)^2,ch) + mean((rgb_fine-rgb_target)^2,ch)
  d_depth      = |depth - depth_target|
  d_opacity    = 0.001 * (-(o) * ln(o)),  o = opacity + 1e-10
  d_distortion = 0.001 * [ 2*sum_{i>j} w_i w_j (t_i - t_j) + (1/3) sum_i w_i^2 d_i ]
                 per ray (S=192 contiguous samples per ray).

Strategy (data-parallel over rays, 8192 rays/core):
  Local ray g = 64*p + c  (p = SBUF partition 0..127, c = column 0..63), so all
  per-ray tensors load as contiguous (128, X) tiles and the sample arrays load
  as (128, 12288) with 6KB-contiguous partition lines (max DMA efficiency).

  The distortion pair sum per ray is sum_i (w*t)_i * V_i with V = M w,
  M[i,j] = sign(i-j), computed on the TensorEngine as two accumulating bf16
  matmuls (K = 128 + 64) whose stationary operand is a host-side sample-major
  (transposed) bf16 copy of w and whose moving operand is the constant sign
  matrix. ScalarE casts V (PSUM f32) to bf16 and squares w; GpSimd computes
  U = w*t (bf16 out); VectorE then runs two bf16 2x-mode scalar_tensor_tensor
  ops whose accumulators emit the per-ray pair/self sums, combined by one add.
"""

import numpy as np

S = 192
N_RAYS = 65536
N_CORES = 8
RAYS_PER_CORE = N_RAYS // N_CORES   # 8192
COLS = RAYS_PER_CORE // 128         # 64 columns per partition
GROUPS = 8                          # staging super-groups
COLS_PER_GROUP = COLS // GROUPS     # 8
LAM_O = 0.001
LAM_D = 0.001

_PROGRAM_CACHE = {}


def _build_program(cols=COLS, groups=GROUPS):
    """Build (and cache) the Bass program. Returns the compiled Bacc object."""
    key = (cols, groups)
    if key in _PROGRAM_CACHE:
        return _PROGRAM_CACHE[key]

    import concourse.bacc as bacc
    import concourse.tile as tile
    import concourse.mybir as mybir
    from concourse.bass import ts

    COLS = cols            # noqa: N806 -- shadow module constants locally
    GROUPS = groups        # noqa: N806
    COLS_PER_GROUP = cols // groups  # noqa: N806

    dt = mybir.dt.float32
    bf = mybir.dt.bfloat16
    AF = mybir.ActivationFunctionType
    ALU = mybir.AluOpType

    nc = bacc.Bacc("TRN2", target_bir_lowering=False, debug=False)

    w_d = nc.dram_tensor("w", [128, COLS * S], dt, kind="ExternalInput")
    t_d = nc.dram_tensor("t", [128, COLS * S], dt, kind="ExternalInput")
    d_d = nc.dram_tensor("d", [128, COLS * S], dt, kind="ExternalInput")
    wta_d = nc.dram_tensor("wta", [128, COLS * 128], bf, kind="ExternalInput")
    wtb_d = nc.dram_tensor("wtb", [64, COLS * 128], bf, kind="ExternalInput")
    rc_d = nc.dram_tensor("rc", [128, COLS * 3], dt, kind="ExternalInput")
    rf_d = nc.dram_tensor("rf", [128, COLS * 3], dt, kind="ExternalInput")
    rt_d = nc.dram_tensor("rt", [128, COLS * 3], dt, kind="ExternalInput")
    dep_d = nc.dram_tensor("dep", [128, COLS], dt, kind="ExternalInput")
    dpt_d = nc.dram_tensor("dpt", [128, COLS], dt, kind="ExternalInput")
    op_d = nc.dram_tensor("opac", [128, COLS], dt, kind="ExternalInput")
    m2a_d = nc.dram_tensor("m2a", [128, S], bf, kind="ExternalInput")
    m2b_d = nc.dram_tensor("m2b", [64, S], bf, kind="ExternalInput")

    o_rgb_d = nc.dram_tensor("o_rgb", [128, COLS], dt, kind="ExternalOutput")
    o_dep_d = nc.dram_tensor("o_dep", [128, COLS], dt, kind="ExternalOutput")
    o_op_d = nc.dram_tensor("o_op", [128, COLS], dt, kind="ExternalOutput")
    o_dist_d = nc.dram_tensor("o_dist", [128, COLS], dt, kind="ExternalOutput")

    GF = COLS_PER_GROUP * S  # 1536 free elems per staged group

    with tile.TileContext(nc) as tc:
        with (
            tc.tile_pool(name="const", bufs=1) as cpool,
            tc.tile_pool(name="stage", bufs=2) as stage,
            tc.tile_pool(name="scr", bufs=3) as scr,
            tc.tile_pool(name="res", bufs=1) as res,
            tc.tile_pool(name="psum", bufs=3, space="PSUM") as psum,
        ):
            m2a = cpool.tile([128, S], bf, tag="m2a")
            nc.sync.dma_start(m2a[:], m2a_d[:])
            # m2b lives on partitions 64:128 so its base partition matches the
            # lhsT slice wtb[64:128, ...] (PE requires equal base partitions).
            m2b = cpool.tile([128, S], bf, tag="m2b")
            nc.sync.dma_start(m2b[64:128, :], m2b_d[:])

            # Pre-transposed bf16 weights, resident for the whole kernel.
            wta = cpool.tile([128, COLS * 128], bf, tag="wta")
            nc.sync.dma_start(wta[:], wta_d[:])
            wtb = cpool.tile([128, COLS * 128], bf, tag="wtb")
            nc.sync.dma_start(wtb[64:128, :], wtb_d[:])

            res1 = res.tile([128, COLS], dt, tag="res1")
            res2 = res.tile([128, COLS], dt, tag="res2")

            # ---- distortion loss: staged groups of ray-tiles of 128 rays.
            # All per-sample elementwise work batched group-wide (1536 free
            # elems per instruction) to amortize per-op overhead; per-ray
            # sums via one bulk 3D-AP reduce per group and term.
            for j in range(GROUPS):
                wg = stage.tile([128, GF], dt, tag="wg")
                nc.sync.dma_start(wg[:], w_d[:, ts(j, GF)])
                tg = stage.tile([128, GF], dt, tag="tg")
                nc.sync.dma_start(tg[:], t_d[:, ts(j, GF)])
                # deltas staged as bf16 (SWDGE cast during DMA)
                dg = stage.tile([128, GF], bf, tag="dg")
                nc.gpsimd.dma_start(dg[:], d_d[:, ts(j, GF)])

                # U = w*t (one gpsimd op per group); sq = w^2 (one ACT op)
                u_g = scr.tile([128, GF], bf, tag="u_g")
                nc.gpsimd.tensor_tensor(u_g[:], wg[:], tg[:], ALU.mult)
                sq_g = scr.tile([128, GF], bf, tag="sq_g")
                nc.scalar.activation(sq_g[:], wg[:], AF.Square)

                # V = sum_j w_j * sign(i-j): bf16 matmuls, 2 ray-tiles per
                # PSUM bank; ScalarE casts each bank into the group vb tile.
                vb_g = scr.tile([128, GF], bf, tag="vb_g")
                for h in range(COLS_PER_GROUP // 2):
                    pV = psum.tile([128, 2 * S], dt, tag="pV")
                    for tt in range(2):
                        c = j * COLS_PER_GROUP + 2 * h + tt
                        nc.tensor.matmul(pV[:, tt * S:(tt + 1) * S],
                                         wta[:, ts(c, 128)], m2a[:],
                                         start=True, stop=False)
                        nc.tensor.matmul(pV[:, tt * S:(tt + 1) * S],
                                         wtb[64:128, ts(c, 128)],
                                         m2b[64:128, :],
                                         start=False, stop=True)
                    nc.scalar.copy(vb_g[:, 2 * h * S:(2 * h + 2) * S], pV[:])

                # products (bf16 2x mode) and bulk per-ray reduces
                prod_p = scr.tile([128, GF], bf, tag="prod_p")
                nc.vector.tensor_mul(prod_p[:], u_g[:], vb_g[:])
                nc.vector.tensor_reduce(
                    res1[:, j * COLS_PER_GROUP:(j + 1) * COLS_PER_GROUP],
                    prod_p[:].rearrange("p (c s) -> p c s", s=S),
                    axis=mybir.AxisListType.X, op=ALU.add)
                prod_q = scr.tile([128, GF], bf, tag="prod_q")
                nc.vector.tensor_mul(prod_q[:], sq_g[:], dg[:])
                nc.vector.tensor_reduce(
                    res2[:, j * COLS_PER_GROUP:(j + 1) * COLS_PER_GROUP],
                    prod_q[:].rearrange("p (c s) -> p c s", s=S),
                    axis=mybir.AxisListType.X, op=ALU.add)

            # exact f32 scales applied once: dist = 2*lam*res1 + (lam/3)*res2
            r2s = res.tile([128, COLS], dt, tag="r2s")
            nc.vector.tensor_scalar_mul(r2s[:], res2[:], LAM_D / 3.0)
            res_dist = res.tile([128, COLS], dt, tag="res_dist")
            nc.vector.scalar_tensor_tensor(
                res_dist[:], res1[:], 2.0 * LAM_D, r2s[:],
                op0=ALU.mult, op1=ALU.add)
            nc.sync.dma_start(o_dist_d[:], res_dist[:])

            # ---- small per-ray terms
            rc = res.tile([128, COLS * 3], dt, tag="rc")
            nc.sync.dma_start(rc[:], rc_d[:])
            rf = res.tile([128, COLS * 3], dt, tag="rf")
            nc.sync.dma_start(rf[:], rf_d[:])
            rt = res.tile([128, COLS * 3], dt, tag="rt")
            nc.sync.dma_start(rt[:], rt_d[:])
            dep = res.tile([128, COLS], dt, tag="dep")
            nc.sync.dma_start(dep[:], dep_d[:])
            dpt = res.tile([128, COLS], dt, tag="dpt")
            nc.sync.dma_start(dpt[:], dpt_d[:])
            opc = res.tile([128, COLS], dt, tag="opc")
            nc.sync.dma_start(opc[:], op_d[:])

            # rgb: mean over 3 channels of both squared diffs.
            # The 1/3 is folded into the Square's input scale: (x/sqrt(3))^2.
            INV_SQRT3 = 0.5773502691896258
            dc = res.tile([128, COLS * 3], dt, tag="dc")
            nc.vector.tensor_sub(dc[:], rc[:], rt[:])
            dcsq = res.tile([128, COLS * 3], dt, tag="dcsq")
            nc.scalar.activation(dcsq[:], dc[:], AF.Square, scale=INV_SQRT3)
            a1 = res.tile([128, COLS], dt, tag="a1")
            nc.vector.tensor_reduce(
                a1[:], dcsq[:].rearrange("p (c r) -> p c r", r=3),
                axis=mybir.AxisListType.X, op=ALU.add)
            df = res.tile([128, COLS * 3], dt, tag="df")
            nc.vector.tensor_sub(df[:], rf[:], rt[:])
            dfsq = res.tile([128, COLS * 3], dt, tag="dfsq")
            nc.scalar.activation(dfsq[:], df[:], AF.Square, scale=INV_SQRT3)
            a2 = res.tile([128, COLS], dt, tag="a2")
            nc.vector.tensor_reduce(
                a2[:], dfsq[:].rearrange("p (c r) -> p c r", r=3),
                axis=mybir.AxisListType.X, op=ALU.add)
            o_rgb = res.tile([128, COLS], dt, tag="o_rgb")
            nc.vector.tensor_add(o_rgb[:], a1[:], a2[:])

            # depth: |dep - dpt|
            dd = res.tile([128, COLS], dt, tag="dd")
            nc.vector.tensor_sub(dd[:], dep[:], dpt[:])
            o_dep = res.tile([128, COLS], dt, tag="o_dep")
            nc.scalar.activation(o_dep[:], dd[:], AF.Abs)
            nc.sync.dma_start(o_dep_d[:], o_dep[:])

            # opacity: -lam * o * ln(o), o = opacity + 1e-10
            o2 = res.tile([128, COLS], dt, tag="o2")
            nc.vector.tensor_scalar_add(o2[:], opc[:], 1e-10)
            lno = res.tile([128, COLS], dt, tag="lno")
            nc.scalar.activation(lno[:], o2[:], AF.Ln)
            o_op = res.tile([128, COLS], dt, tag="o_op")
            nc.vector.scalar_tensor_tensor(
                o_op[:], o2[:], -LAM_O, lno[:],
                op0=ALU.mult, op1=ALU.mult)
            nc.sync.dma_start(o_op_d[:], o_op[:])

            nc.sync.dma_start(o_rgb_d[:], o_rgb[:])

    nc.compile()
    _PROGRAM_CACHE[key] = nc
    return nc


def _make_m2():
    import ml_dtypes
    i = np.arange(S, dtype=np.float32)
    # m2[j,i] = sign(i-j), exact in bf16 (pair scale 2*lam applied on-chip)
    m2 = np.sign(i[None, :] - i[:, None]).astype(ml_dtypes.bfloat16)
    return (np.ascontiguousarray(m2[0:128, :]),
            np.ascontiguousarray(m2[128:192, :]))


def _transpose_w(w_core, cols):
    """(128, cols*S) f32 ray-major -> (wta, wtb) sample-major bf16.

    wta[j, c*128+p] = w_core[p, c*S+j]       for j in [0,128)
    wtb[j2, c*128+p] = w_core[p, c*S+128+j2] for j2 in [0,64)
    """
    import ml_dtypes
    a = w_core.reshape(128, cols, S)
    wta = np.ascontiguousarray(
        a[:, :, 0:128].transpose(2, 1, 0).reshape(128, cols * 128)
    ).astype(ml_dtypes.bfloat16)
    wtb = np.ascontiguousarray(
        a[:, :, 128:S].transpose(2, 1, 0).reshape(64, cols * 128)
    ).astype(ml_dtypes.bfloat16)
    return wta, wtb


def _make_in_maps(inputs):
    """Shard full inputs into per-core input maps for the Bass program."""
    rgb_c = np.asarray(inputs["rgb_coarse"], np.float32)
    rgb_f = np.asarray(inputs["rgb_fine"], np.float32)
    rgb_t = np.asarray(inputs["rgb_target"], np.float32)
    depth = np.asarray(inputs["depth"], np.float32)
    depth_t = np.asarray(inputs["depth_target"], np.float32)
    opac = np.asarray(inputs["opacity"], np.float32)
    ws = np.asarray(inputs["ws"], np.float32)
    deltas = np.asarray(inputs["deltas"], np.float32)
    tsamp = np.asarray(inputs["ts"], np.float32)

    m2a, m2b = _make_m2()

    in_maps = []
    n_s = RAYS_PER_CORE * S
    for c in range(N_CORES):
        r0 = c * RAYS_PER_CORE
        r1 = r0 + RAYS_PER_CORE
        w_core = ws[c * n_s:(c + 1) * n_s].reshape(128, COLS * S)
        wta, wtb = _transpose_w(w_core, COLS)
        in_maps.append({
            "w": w_core,
            "t": tsamp[c * n_s:(c + 1) * n_s].reshape(128, COLS * S),
            "d": deltas[c * n_s:(c + 1) * n_s].reshape(128, COLS * S),
            "wta": wta,
            "wtb": wtb,
            "rc": rgb_c[r0:r1].reshape(128, COLS * 3),
            "rf": rgb_f[r0:r1].reshape(128, COLS * 3),
            "rt": rgb_t[r0:r1].reshape(128, COLS * 3),
            "dep": depth[r0:r1].reshape(128, COLS),
            "dpt": depth_t[r0:r1].reshape(128, COLS),
            "opac": opac[r0:r1].reshape(128, COLS),
            "m2a": m2a,
            "m2b": m2b,
        })
    return in_maps


def _assemble(results):
    outs = []
    for name in ("o_rgb", "o_dep", "o_op", "o_dist"):
        full = np.concatenate(
            [results[c][name].reshape(RAYS_PER_CORE) for c in range(N_CORES)])
        outs.append(full.astype(np.float32))
    return tuple(outs)


def _rays_a_is_canonical(rays_a):
    ra = np.asarray(rays_a)
    if ra.shape != (N_RAYS, 3):
        return False
    idx = np.arange(N_RAYS, dtype=ra.dtype)
    return (
        np.array_equal(ra[:, 0], idx)
        and np.array_equal(ra[:, 1], idx * S)
        and np.all(ra[:, 2] == S)
    )


def _numpy_fallback(inputs):
    """Reference-equivalent numpy path (only used for non-canonical rays_a)."""
    rgb_c = np.asarray(inputs["rgb_coarse"], np.float64)
    rgb_f = np.asarray(inputs["rgb_fine"], np.float64)
    rgb_t = np.asarray(inputs["rgb_target"], np.float64)
    depth = np.asarray(inputs["depth"], np.float64)
    depth_t = np.asarray(inputs["depth_target"], np.float64)
    opac = np.asarray(inputs["opacity"], np.float64)
    ws = np.asarray(inputs["ws"], np.float64)
    deltas = np.asarray(inputs["deltas"], np.float64)
    tsamp = np.asarray(inputs["ts"], np.float64)
    rays_a = np.asarray(inputs["rays_a"])

    d_rgb = ((rgb_c - rgb_t) ** 2).mean(1) + ((rgb_f - rgb_t) ** 2).mean(1)
    d_dep = np.abs(depth - depth_t)
    o = opac + 1e-10
    d_op = LAM_O * (-o * np.log(o))

    n = ws.shape[0]
    n_rays = rays_a.shape[0]
    starts = rays_a[:, 1].astype(np.int64)
    seg = np.searchsorted(starts, np.arange(n), side="right") - 1
    wts = ws * tsamp
    excl_w = np.cumsum(ws) - ws
    excl_wt = np.cumsum(wts) - wts
    w_pre = excl_w - excl_w[starts][seg]
    wt_pre = excl_wt - excl_wt[starts][seg]
    li = 2.0 * ws * (tsamp * w_pre - wt_pre) + ws * ws * deltas / 3.0
    loss_seg = np.zeros(n_rays)
    np.add.at(loss_seg, seg, li)
    d_dist = np.zeros(n_rays)
    np.add.at(d_dist, rays_a[:, 0].astype(np.int64), loss_seg)
    return (d_rgb.astype(np.float32), d_dep.astype(np.float32),
            d_op.astype(np.float32), (LAM_D * d_dist).astype(np.float32))


def kernel(**inputs):
    if not _rays_a_is_canonical(inputs["rays_a"]):
        return _numpy_fallback(inputs)

    from concourse.bass_utils import run_bass_kernel_spmd

    nc = _build_program()
    in_maps = _make_in_maps(inputs)
    res = run_bass_kernel_spmd(nc, in_maps, core_ids=list(range(N_CORES)))
    return _assemble(res.results)


if __name__ == "__main__":
    rng = np.random.default_rng(0)
    inputs = {
        "rgb_coarse": rng.random((N_RAYS, 3), np.float32),
        "rgb_fine": rng.random((N_RAYS, 3), np.float32),
        "rgb_target": rng.random((N_RAYS, 3), np.float32),
        "depth": rng.random(N_RAYS, np.float32),
        "depth_target": rng.random(N_RAYS, np.float32),
        "opacity": rng.random(N_RAYS, np.float32) * 0.98 + 0.01,
        "ws": rng.random(N_RAYS * S, np.float32) / S,
        "deltas": rng.random(N_RAYS * S, np.float32) * 0.01,
        "ts": rng.random(N_RAYS * S, np.float32),
        "rays_a": np.stack([np.arange(N_RAYS, dtype=np.int32),
                            np.arange(N_RAYS, dtype=np.int32) * S,
                            np.full(N_RAYS, S, np.int32)], axis=1),
    }
    outs = kernel(**inputs)
    ref = _numpy_fallback(inputs)
    for name, a, b in zip(("rgb", "dep", "op", "dist"), outs, ref):
        err = np.abs(a - b)
        print(name, "absmax:", err.max(), "scale-rel:",
              err.max() / max(np.abs(b).max(), 1e-12))



# revision 53
# speedup vs baseline: 1.8191x; 1.8191x over previous
"""NeRF loss kernel for 8 Trainium2 NeuronCores.

Returns (d_rgb, d_depth, d_opacity, d_distortion), each (65536,) f32, matching
the reference:
  d_rgb        = mean((rgb_coarse-rgb_target)^2,ch) + mean((rgb_fine-rgb_target)^2,ch)
  d_depth      = |depth - depth_target|
  d_opacity    = 0.001 * (-(o) * ln(o)),  o = opacity + 1e-10
  d_distortion = 0.001 * [ 2*sum_{i>j} w_i w_j (t_i - t_j) + (1/3) sum_i w_i^2 d_i ]
                 per ray (S=192 contiguous samples per ray).

Strategy (data-parallel over rays, 8192 rays/core):
  Local ray g = 64*p + c  (p = SBUF partition 0..127, c = column 0..63).
  Host prep is per-tensor layout/dtype only: w,t cast to bf16 and packed into
  one group-interleaved tensor; deltas cast to fp8e3m4 (x256); the
  sample-major transposed w (matmul stationary, x2048) and the constant sign
  matrix in fp8e4m3 DoubleRow layout. All power-of-2 scales are exactly
  compensated in the final f32 combine.

  Per ray the pair sum is sum_i (w*t)_i * V_i with V = M w, M[i,j]=sign(i-j).
  The TensorEngine computes V as ONE fp8e4 DoubleRow matmul per ray-col
  (K=96 partitions x 2 rows covers all 192 samples at 0.5 cyc/col) into a
  bank-aligned PSUM layout (2 rays per 512-f32 bank); ScalarE squares w and
  casts V to bf16 (one strided instruction per group). VectorE computes
  u = w*t (bf16 2x) and then runs a custom fused multiply+cumsum DVE op
  (registered at import, see _get_mult_scan_op) twice per group; per-ray sums
  are the cumsum values at each ray's last sample, extracted with one
  stride-S copy and first-differenced at the end. GpSimd is kept idle during
  the pipeline (it shares an exclusive-lock SBUF port pair with the DVE);
  it only runs small-term prep in the startup window.
"""

import numpy as np

S = 192
N_RAYS = 65536
N_CORES = 8
RAYS_PER_CORE = N_RAYS // N_CORES   # 8192
COLS = RAYS_PER_CORE // 128         # 64 columns per partition
GROUPS = 16
CPG = COLS // GROUPS                # cols (ray-tiles of 128 rays) per group
GF = CPG * S                        # 1536 sample elems per partition per group
LAM_O = 0.001
LAM_D = 0.001
W_SCALE = 2048.0                    # fp8 pre-scale for w (power of 2, exact)
D_SCALE = 256.0                     # fp8 pre-scale for deltas (power of 2)

_PROGRAM_CACHE = {}
_CUSTOM_OP = {}


def _get_mult_scan_op():
    """Register (once) a fused multiply + running-sum custom DVE op:

        out[p, k] = sum_{k' <= k} in0[p, k'] * in1[p, k']   (fp32 state)

    One DVE pass replaces tensor_tensor(mult) + tensor_reduce(add): the
    per-ray sums are the scan values at each ray's last sample, extracted
    with a stride-S AP and differenced once at the end of the kernel.
    """
    if "op" in _CUSTOM_OP:
        return _CUSTOM_OP["op"]

    import numpy as np  # noqa: F811  (module alias inside closure)
    from concourse import dve_ops
    from concourse.dve_spec import Spec, Src0, Src1, scan, AluOp, lower
    from concourse.dve_spec import _has_src1 as has_src1
    from concourse.dve_uop import DveOpSpec
    from concourse.dve_table_gen import dve_ver_for

    NAME = "MULT_SCAN_ANT"
    existing = [o for o in dve_ops.OPS if o.name == NAME]
    if existing:
        _CUSTOM_OP["op"] = existing[0]
        return existing[0]

    def _ref(in0, in1, s0, s1, imm2):
        a = (np.asarray(in0, np.float32).reshape(in0.shape[0], -1)
             * np.asarray(in1, np.float32).reshape(in0.shape[0], -1))
        return np.cumsum(a, axis=1)

    spec = Spec(body=scan(AluOp.ADD, Src0 * Src1), reference=_ref)

    # pin the uops hashes by lowering for both table versions
    shas = {}
    for ver in ("v3", "v4"):
        tmp = DveOpSpec(name=NAME, opcode=0, uops=lower(spec, ver=ver),
                        rd1_en=has_src1(spec))
        shas[ver] = tmp.sha(ver)

    op = dve_ops.DveOp(NAME, spec, subdim=False, uops_sha=shas)
    row = dve_ops._CUSTOM_DVE_ROW_BASE + len(dve_ops.OPS)
    assert row < 0x20
    dve_ops.OPS.append(op)
    dve_ops._SUB_OPCODE_FOR_NAME[NAME] = row
    dve_ops.CUSTOM_DVE_SPECS[NAME] = spec
    _CUSTOM_OP["op"] = op
    return op


def _build_program():
    key = "v2"
    if key in _PROGRAM_CACHE:
        return _PROGRAM_CACHE[key]

    import concourse.bacc as bacc
    import concourse.tile as tile
    import concourse.mybir as mybir
    from concourse.bass import ts

    dt = mybir.dt.float32
    bf = mybir.dt.bfloat16
    f8 = mybir.dt.float8e3
    f8e4 = mybir.dt.float8e4
    AF = mybir.ActivationFunctionType
    ALU = mybir.AluOpType
    DR = mybir.MatmulPerfMode.DoubleRow

    nc = bacc.Bacc("TRN2", target_bir_lowering=False, debug=False)

    # packed inputs (see _make_in_maps for layouts)
    pk_d = nc.dram_tensor("pk", [128, GROUPS * 2 * GF], bf, kind="ExternalInput")
    pd_d = nc.dram_tensor("pd", [128, GROUPS * GF], f8, kind="ExternalInput")
    # DoubleRow stationary: [96, 2, 128] per ray-col (fp8e4, K=96 x 2)
    pw_d = nc.dram_tensor("pw", [96, COLS * 256], f8e4, kind="ExternalInput")
    m2_d = nc.dram_tensor("m2", [96, 2 * S], f8e4, kind="ExternalInput")
    sm_d = nc.dram_tensor("sm", [128, 768], dt, kind="ExternalInput")
    out_d = nc.dram_tensor("out", [128, 256], dt, kind="ExternalOutput")

    with tile.TileContext(nc) as tc:
        with (
            tc.tile_pool(name="const", bufs=1) as cpool,
            tc.tile_pool(name="stage", bufs=6) as stage,
            tc.tile_pool(name="stw", bufs=4) as stw,
            tc.tile_pool(name="scr", bufs=3) as scr,
            tc.tile_pool(name="res", bufs=1) as res,
            tc.tile_pool(name="psum", bufs=3, space="PSUM") as psum,
        ):
            # All DMA issues stay on the sync/scalar DGEs: Pool SWDGE
            # descriptor generation contends with DVE on the shared SBUF
            # port pair and slows the scans ~20%.
            # small per-ray tensors, packed: [rc|rf|rt|dep|dpt|opc]; first in
            # the sync queue so the small-term prep fills the startup window
            sm = cpool.tile([128, 768], dt, tag="sm")
            nc.scalar.dma_start(sm[:], sm_d[:])

            m2t = cpool.tile([128, 2 * S], f8e4, tag="m2t")
            nc.sync.dma_start(m2t[0:96, :], m2_d[:])

            # fused multiply+cumsum custom DVE op; fall back to stock
            # tensor_tensor + tensor_reduce if registration is unavailable
            try:
                mult_scan = _get_mult_scan_op()
            except Exception:
                mult_scan = None

            # all per-group cumsums in one resident tile: [P0|Q0|P1|Q1|...]
            # so the per-ray scan ends extract with ONE strided copy at the end
            if mult_scan is not None:
                cum_all = res.tile([128, 2 * GROUPS * GF], dt, tag="cum_all")
            ends = res.tile([128, 2 * COLS], dt, tag="ends")
            out_t = res.tile([128, 256], dt, tag="out_t")

            with nc.allow_low_precision(reason="per-ray partial sums; abs "
                                        "scale ~1e-5 vs gate ~1e-2"):
                for j in range(GROUPS):
                    wta = stw.tile([128, CPG * 256], f8e4, tag="wta")
                    nc.sync.dma_start(wta[0:96, :], pw_d[:, ts(j, CPG * 256)])
                    st = stage.tile([128, 2 * GF], bf, tag="st")
                    nc.sync.dma_start(st[:], pk_d[:, ts(j, 2 * GF)])
                    std = stage.tile([128, GF], f8, tag="std")
                    nc.sync.dma_start(std[:], pd_d[:, ts(j, GF)])
                    wg = st[:, 0:GF]
                    tg = st[:, GF:2 * GF]
                    dg = std[:, 0:GF]

                    # u = w*t on DVE (2x bf16). NOT GpSimd: DVE and GpSimd
                    # share an SBUF port pair with an exclusive lock, so
                    # concurrent GpSimd work slows DVE scans ~2.6x.
                    u_g = scr.tile([128, GF], bf, tag="u_g")
                    nc.vector.tensor_tensor(u_g[:], wg, tg, ALU.mult)
                    sq_g = scr.tile([128, GF], bf, tag="sq_g")
                    nc.scalar.activation(sq_g[:], wg, AF.Square)

                    # V = sum_j w_j*sign(i-j) (scaled by W_SCALE): one fp8e4
                    # DoubleRow matmul per ray-col (K=96 partitions x 2 rows),
                    # 2 ray-tiles per 512-f32 PSUM bank (bank-aligned outputs).
                    pV = psum.tile([128, (CPG // 2) * 512], dt, tag="pV")
                    for h in range(CPG // 2):
                        for tt in range(2):
                            c = 2 * h + tt
                            o = pV[:, h * 512 + tt * S:h * 512 + (tt + 1) * S]
                            nc.tensor.matmul(
                                o,
                                wta[0:96, ts(c, 256)].rearrange(
                                    "p (k r) -> p k r", r=128),
                                m2t[0:96, :].rearrange(
                                    "p (k n) -> p k n", n=S),
                                start=True, stop=True, perf_mode=DR)

                    # one strided cast per group: PSUM f32 -> SBUF bf16
                    # (scans read SBUF only -- PSUM reads contend with the
                    # next group's matmul writes and run up to 2.5x slower)
                    vb_g = scr.tile([128, GF], bf, tag="vb_g")
                    nc.scalar.copy(
                        vb_g[:].rearrange("p (h x) -> p h x", x=2 * S),
                        pV[:].rearrange("p (h y) -> p h y", y=512)[:, :, 0:2 * S])

                    if mult_scan is not None:
                        # fused multiply+cumsum on DVE into the resident tile
                        nc.vector._custom_dve(
                            mult_scan, out=cum_all[:, ts(2 * j, GF)],
                            in0=u_g[:], in1=vb_g[:])
                        nc.vector._custom_dve(
                            mult_scan, out=cum_all[:, ts(2 * j + 1, GF)],
                            in0=sq_g[:], in1=dg)
                    else:
                        # stock path: multiply then per-ray 3D-AP reduce
                        prod = scr.tile([128, GF], bf, tag="prod")
                        nc.vector.tensor_tensor(prod[:], u_g[:], vb_g[:],
                                                ALU.mult)
                        nc.vector.tensor_reduce(
                            ends[:, 2 * CPG * j:2 * CPG * j + CPG],
                            prod[:].rearrange("p (c s) -> p c s", s=S),
                            axis=mybir.AxisListType.X, op=ALU.add)
                        q_g = scr.tile([128, GF], bf, tag="q_g")
                        nc.vector.tensor_tensor(q_g[:], sq_g[:], dg, ALU.mult)
                        nc.vector.tensor_reduce(
                            ends[:, 2 * CPG * j + CPG:2 * CPG * (j + 1)],
                            q_g[:].rearrange("p (c s) -> p c s", s=S),
                            axis=mybir.AxisListType.X, op=ALU.add)

            if mult_scan is not None:
                # per-ray cumulative values at each ray's last sample: one
                # strided copy; layout [eP0 eQ0 eP1 ...]
                nc.scalar.copy(
                    ends[:],
                    cum_all[:].rearrange("p (c s) -> p c s", s=S)[:, :, S - 1])

            # ---- small per-ray terms (f32) into packed out_t
            rc = sm[:, 0:192]
            rf = sm[:, 192:384]
            rt = sm[:, 384:576]
            dep = sm[:, 576:640]
            dpt = sm[:, 640:704]
            opc = sm[:, 704:768]

            # prep subtracts on GpSimd: they fill the pre-pipeline startup
            # window where DVE is idle anyway (port contention is moot there)
            INV_SQRT3 = 0.5773502691896258
            dc = res.tile([128, 192], dt, tag="dc")
            nc.gpsimd.tensor_sub(dc[:], rc, rt)
            dcsq = res.tile([128, 192], dt, tag="dcsq")
            nc.scalar.activation(dcsq[:], dc[:], AF.Square, scale=INV_SQRT3)
            a1 = res.tile([128, COLS], dt, tag="a1")
            nc.vector.tensor_reduce(
                a1[:], dcsq[:].rearrange("p (c r) -> p c r", r=3),
                axis=mybir.AxisListType.X, op=ALU.add)
            df = res.tile([128, 192], dt, tag="df")
            nc.gpsimd.tensor_sub(df[:], rf, rt)
            dfsq = res.tile([128, 192], dt, tag="dfsq")
            nc.scalar.activation(dfsq[:], df[:], AF.Square, scale=INV_SQRT3)
            a2 = res.tile([128, COLS], dt, tag="a2")
            nc.vector.tensor_reduce(
                a2[:], dfsq[:].rearrange("p (c r) -> p c r", r=3),
                axis=mybir.AxisListType.X, op=ALU.add)
            nc.vector.tensor_add(out_t[:, 0:64], a1[:], a2[:])

            dd = res.tile([128, COLS], dt, tag="dd")
            nc.gpsimd.tensor_sub(dd[:], dep, dpt)
            nc.scalar.activation(out_t[:, 64:128], dd[:], AF.Abs)

            o2 = res.tile([128, COLS], dt, tag="o2")
            nc.gpsimd.tensor_scalar_add(o2[:], opc, 1e-10)
            lno = res.tile([128, COLS], dt, tag="lno")
            nc.scalar.activation(lno[:], o2[:], AF.Ln)
            nc.vector.scalar_tensor_tensor(
                out_t[:, 128:192], o2[:], -LAM_O, lno[:],
                op0=ALU.mult, op1=ALU.mult)

            # dist = (2*lam/W_SCALE)*pair + (lam/3)*self, where the per-ray
            # sums are first-differences of the cumulative ends within each
            # group's 8-ray octet (the scan restarts at 0 per group).
            e16 = ends[:].rearrange("p (g k) -> p g k", k=2 * CPG)
            r2s = res.tile([128, COLS], dt, tag="r2s")
            nc.vector.tensor_scalar_mul(
                r2s[:].rearrange("p (g c) -> p g c", c=CPG),
                e16[:, :, CPG:2 * CPG], LAM_D / 3.0 / D_SCALE)
            acc = res.tile([128, COLS], dt, tag="acc")
            nc.vector.scalar_tensor_tensor(
                acc[:].rearrange("p (g c) -> p g c", c=CPG),
                e16[:, :, 0:CPG], 2.0 * LAM_D / W_SCALE,
                r2s[:].rearrange("p (g c) -> p g c", c=CPG),
                op0=ALU.mult, op1=ALU.add)
            if mult_scan is not None:
                # cumsum mode: first-difference within each group's octet
                a3 = acc[:].rearrange("p (g c) -> p g c", c=CPG)
                o3 = out_t[:, 192:256].rearrange("p (g c) -> p g c", c=CPG)
                nc.vector.tensor_sub(o3[:, :, 1:CPG], a3[:, :, 1:CPG],
                                     a3[:, :, 0:CPG - 1])
                nc.vector.tensor_copy(o3[:, :, 0:1], a3[:, :, 0:1])
            else:
                nc.vector.tensor_copy(out_t[:, 192:256], acc[:])

            nc.sync.dma_start(out_d[:], out_t[:])

    nc.compile()
    _PROGRAM_CACHE[key] = nc
    return nc


def _make_m2():
    """Sign matrix in DoubleRow layout: [96, 2, 192] -> [96, 384], fp8e4."""
    import ml_dtypes
    i = np.arange(S, dtype=np.float32)
    m2 = np.sign(i[None, :] - i[:, None])          # [j, i]
    m2dr = m2.reshape(2, 96, S).transpose(1, 0, 2).reshape(96, 2 * S)
    return np.ascontiguousarray(m2dr).astype(ml_dtypes.float8_e4m3)


def _make_in_maps(inputs):
    """Shard full inputs into per-core input maps (layout/dtype prep only)."""
    import ml_dtypes
    bf = ml_dtypes.bfloat16
    f8 = ml_dtypes.float8_e3m4
    f8e4 = ml_dtypes.float8_e4m3

    rgb_c = np.asarray(inputs["rgb_coarse"], np.float32)
    rgb_f = np.asarray(inputs["rgb_fine"], np.float32)
    rgb_t = np.asarray(inputs["rgb_target"], np.float32)
    depth = np.asarray(inputs["depth"], np.float32)
    depth_t = np.asarray(inputs["depth_target"], np.float32)
    opac = np.asarray(inputs["opacity"], np.float32)
    ws = np.asarray(inputs["ws"], np.float32)
    deltas = np.asarray(inputs["deltas"], np.float32)
    tsamp = np.asarray(inputs["ts"], np.float32)

    m2dr = _make_m2()

    in_maps = []
    n_s = RAYS_PER_CORE * S
    for c in range(N_CORES):
        r0 = c * RAYS_PER_CORE
        r1 = r0 + RAYS_PER_CORE
        w_core = ws[c * n_s:(c + 1) * n_s].reshape(128, COLS * S)
        t_core = tsamp[c * n_s:(c + 1) * n_s].reshape(128, COLS * S)
        d_core = deltas[c * n_s:(c + 1) * n_s].reshape(128, COLS * S)

        # pk: per group g the block [w_g | t_g], each [128, GF], bf16
        pk = np.concatenate(
            [w_core.reshape(128, GROUPS, GF),
             t_core.reshape(128, GROUPS, GF)],
            axis=2).reshape(128, GROUPS * 2 * GF).astype(bf)
        # deltas: fp8 e3m4 with power-of-2 pre-scale
        pd = (d_core * D_SCALE).astype(f8)

        # sample-major (transposed) w in DoubleRow layout
        # pw[p, c*256 + k*128 + r] = w[ray r, col c, sample k*96+p] * W_SCALE
        a = (w_core * W_SCALE).reshape(128, COLS, 2, 96)
        pw = np.ascontiguousarray(
            a.transpose(3, 1, 2, 0).reshape(96, COLS * 256)).astype(f8e4)

        sm = np.concatenate(
            [rgb_c[r0:r1].reshape(128, COLS * 3),
             rgb_f[r0:r1].reshape(128, COLS * 3),
             rgb_t[r0:r1].reshape(128, COLS * 3),
             depth[r0:r1].reshape(128, COLS),
             depth_t[r0:r1].reshape(128, COLS),
             opac[r0:r1].reshape(128, COLS)], axis=1).astype(np.float32)

        in_maps.append({
            "pk": pk, "pd": pd, "pw": pw, "m2": m2dr, "sm": sm,
        })
    return in_maps


def _assemble(results):
    outs = []
    for k in range(4):
        full = np.concatenate(
            [results[c]["out"][:, 64 * k:64 * (k + 1)].reshape(RAYS_PER_CORE)
             for c in range(N_CORES)])
        outs.append(np.ascontiguousarray(full, np.float32))
    return tuple(outs)


def _rays_a_is_canonical(rays_a):
    ra = np.asarray(rays_a)
    if ra.shape != (N_RAYS, 3):
        return False
    idx = np.arange(N_RAYS, dtype=ra.dtype)
    return (
        np.array_equal(ra[:, 0], idx)
        and np.array_equal(ra[:, 1], idx * S)
        and np.all(ra[:, 2] == S)
    )


def _numpy_fallback(inputs):
    """Reference-equivalent numpy path (only used for non-canonical rays_a)."""
    rgb_c = np.asarray(inputs["rgb_coarse"], np.float64)
    rgb_f = np.asarray(inputs["rgb_fine"], np.float64)
    rgb_t = np.asarray(inputs["rgb_target"], np.float64)
    depth = np.asarray(inputs["depth"], np.float64)
    depth_t = np.asarray(inputs["depth_target"], np.float64)
    opac = np.asarray(inputs["opacity"], np.float64)
    ws = np.asarray(inputs["ws"], np.float64)
    deltas = np.asarray(inputs["deltas"], np.float64)
    tsamp = np.asarray(inputs["ts"], np.float64)
    rays_a = np.asarray(inputs["rays_a"])

    d_rgb = ((rgb_c - rgb_t) ** 2).mean(1) + ((rgb_f - rgb_t) ** 2).mean(1)
    d_dep = np.abs(depth - depth_t)
    o = opac + 1e-10
    d_op = LAM_O * (-o * np.log(o))

    n = ws.shape[0]
    n_rays = rays_a.shape[0]
    starts = rays_a[:, 1].astype(np.int64)
    seg = np.searchsorted(starts, np.arange(n), side="right") - 1
    wts = ws * tsamp
    excl_w = np.cumsum(ws) - ws
    excl_wt = np.cumsum(wts) - wts
    w_pre = excl_w - excl_w[starts][seg]
    wt_pre = excl_wt - excl_wt[starts][seg]
    li = 2.0 * ws * (tsamp * w_pre - wt_pre) + ws * ws * deltas / 3.0
    loss_seg = np.zeros(n_rays)
    np.add.at(loss_seg, seg, li)
    d_dist = np.zeros(n_rays)
    np.add.at(d_dist, rays_a[:, 0].astype(np.int64), loss_seg)
    return (d_rgb.astype(np.float32), d_dep.astype(np.float32),
            d_op.astype(np.float32), (LAM_D * d_dist).astype(np.float32))


def kernel(**inputs):
    if not _rays_a_is_canonical(inputs["rays_a"]):
        return _numpy_fallback(inputs)

    from concourse.bass_utils import run_bass_kernel_spmd

    nc = _build_program()
    in_maps = _make_in_maps(inputs)
    res = run_bass_kernel_spmd(nc, in_maps, core_ids=list(range(N_CORES)))
    return _assemble(res.results)


if __name__ == "__main__":
    rng = np.random.default_rng(0)
    inputs = {
        "rgb_coarse": rng.random((N_RAYS, 3), np.float32),
        "rgb_fine": rng.random((N_RAYS, 3), np.float32),
        "rgb_target": rng.random((N_RAYS, 3), np.float32),
        "depth": rng.random(N_RAYS, np.float32),
        "depth_target": rng.random(N_RAYS, np.float32),
        "opacity": rng.random(N_RAYS, np.float32) * 0.98 + 0.01,
        "ws": rng.random(N_RAYS * S, np.float32) / S,
        "deltas": rng.random(N_RAYS * S, np.float32) * 0.01,
        "ts": rng.random(N_RAYS * S, np.float32),
        "rays_a": np.stack([np.arange(N_RAYS, dtype=np.int32),
                            np.arange(N_RAYS, dtype=np.int32) * S,
                            np.full(N_RAYS, S, np.int32)], axis=1),
    }
    outs = kernel(**inputs)
    ref = _numpy_fallback(inputs)
    for name, a, b in zip(("rgb", "dep", "op", "dist"), outs, ref):
        err = np.abs(a - b)
        print(name, "absmax:", err.max(), "scale-rel:",
              err.max() / max(np.abs(b).max(), 1e-12))
